# revision 80
# baseline (speedup 1.0000x reference)
"""Multi-head attention (B=2, S=4096, HIDDEN=512, HEADS=8) on 8 TRN2 NeuronCores.

Sharding: 8 cores = 2 batches x 4 head-groups (2 heads each).
Core c handles batch b = c//4 and heads {2g, 2g+1} where g = c%4
(projection feature slice [g*128, (g+1)*128)).

Per-core kernel v2 (mixed precision, three-engine softmax):
  - x^T and all weights arrive from the host pre-converted to bf16;
    Q^T/K^T projections run in bf16 (f32 psum) with the f32 bias added on
    DVE while converting to bf16 kt/qt tiles
  - V is produced directly in natural [t, d] layout (x^T chunks as the
    stationary operand), bias via a K=1 ones-row matmul into the psum
    accumulator, then quantized to fp8e4 vp tiles shaped [128, 2, 65]
    (two key-blocks side by side + a ones column for the softmax
    denominator)
  - scores S^T = K^T.T Q^T in bf16, two key blocks per psum group
  - softmax exp is split across three engines per a static schedule:
    ACT runs the exact table exp (psum f32 -> fp8e4), DVE and GPSIMD run
    a Schraudolph bit-trick exp (tensor_scalar f32->int32 with the magic
    scale/bias, bitcast, convert to fp8e4).  The ~3% sawtooth error of
    the bit-trick largely cancels in the softmax normalization.
  - PV uses fp8e4 DoubleRow matmuls: one instruction contracts both key
    blocks of a pair, so a whole 512-query x 256-key x 65-col update
    costs 256 PE cycles; the ones column accumulates the denominator l
  - normalization: r = 1/l via the exact DVE reciprocal (the approx
    custom op miscomputes on hardware), broadcast across partitions on
    GPSIMD, multiplied in while converting ctx to bf16 (head 1 shifted
    to partitions 64-127 via two quadrant copies)
  - output projection in bf16 against this core's Wo rows -> partial
    [S, 512]; host sums the 4 partials per batch and adds bo.
"""

import sys

import numpy as np

B, S, HID, HEADS, HD = 2, 4096, 512, 8, 64
FSL = 128          # features per core = 2 heads * 64
NCORES = 8
QC = 512           # query-chunk width
NTB = S // 128     # 32 key blocks
NTB2 = NTB // 2    # 16 key-block pairs
NQC = S // QC      # 8 query chunks

# Schraudolph exp constants (includes the 1/sqrt(HD)=0.125 score scale).
_LOG2E = 1.4426950408889634
SCALE = 0.125
A_CONST = float(np.float32(SCALE * _LOG2E * (1 << 23)))
B_CONST = float(np.float32((127.0 - 0.0434609) * (1 << 23)))

# Per-qc schedule for the 32 attention units (16 tb2-pairs x 2 heads).
# 'F' units run both key blocks as one ACT exp out of the wide "st" psum
# ring; 'S' units split into two half-groups on the narrow "sth" ring,
# each half exp'd via the Schraudolph bit-trick on DVE ('D') or GPSIMD
# ('P').  The two rings decouple the ACT pipeline from the slower
# DVE/GPSIMD one.
_S_POS = {2, 5, 7, 10, 13, 15, 18, 21, 23, 26, 29, 31}
UNIT_PATTERN = ["S" if _i in _S_POS else "F" for _i in range(32)]

_PROGRAM = None


def _ensure_imports():
    try:
        import concourse  # noqa: F401
    except ImportError:
        sys.path.insert(0, "/opt/trn_rl_repo")


def _build_program():
    _ensure_imports()
    import concourse.bacc as bacc
    import concourse.mybir as mybir
    import concourse.tile as tile

    f32 = mybir.dt.float32
    bf16 = mybir.dt.bfloat16
    fp8 = mybir.dt.float8e4
    i32 = mybir.dt.int32
    AF = mybir.ActivationFunctionType
    DR = mybir.MatmulPerfMode.DoubleRow
    MULT = mybir.AluOpType.mult
    ADD = mybir.AluOpType.add

    nc = bacc.Bacc(
        "TRN2",
        target_bir_lowering=False,
        debug=False,
        enable_asserts=False,
        num_devices=NCORES,
    )

    xT = nc.dram_tensor("xT", [HID, S], bf16, kind="ExternalInput").ap()
    wqT = nc.dram_tensor("wqT", [128, 4, FSL], bf16, kind="ExternalInput").ap()
    wkT = nc.dram_tensor("wkT", [128, 4, FSL], bf16, kind="ExternalInput").ap()
    wvT = nc.dram_tensor("wvT", [128, 4, FSL], bf16, kind="ExternalInput").ap()
    woT = nc.dram_tensor("woT", [FSL, HID], bf16, kind="ExternalInput").ap()
    bq = nc.dram_tensor("bq", [FSL, 1], f32, kind="ExternalInput").ap()
    bk = nc.dram_tensor("bk", [FSL, 1], f32, kind="ExternalInput").ap()
    bvT = nc.dram_tensor("bvT", [1, FSL], bf16, kind="ExternalInput").ap()
    out = nc.dram_tensor("out", [S, HID], f32, kind="ExternalOutput").ap()

    with tile.TileContext(nc) as tc:
        with (
            tc.tile_pool(name="persist", bufs=1) as pp,
            tc.tile_pool(name="vp_pool", bufs=2 * NTB2) as vpp,
        ):
            kt = pp.tile([FSL, S], bf16, tag="kt")
            qt = pp.tile([FSL, S], bf16, tag="qt")
            ctx01 = pp.tile([FSL, S], bf16, tag="ctx01")
            ctx1b = pp.tile([HD, S], bf16, tag="ctx1b")
            wo_sb = pp.tile([FSL, HID], bf16, tag="wo_sb")
            wo2_sb = pp.tile([HD, HID], bf16, tag="wo2_sb")
            bq_sb = pp.tile([FSL, 1], f32, tag="bq_sb")
            bk_sb = pp.tile([FSL, 1], f32, tag="bk_sb")
            bvT_sb = pp.tile([1, FSL], bf16, tag="bvT_sb")
            ones_row = pp.tile([1, FSL], bf16, tag="ones_row")
            warm = pp.tile([128, 1], f32, tag="warm")

            # biases ride the SWDGE ring so they don't occupy the HWDGE
            # rings' heads
            nc.gpsimd.dma_start(bq_sb[:], bq[:])
            nc.gpsimd.dma_start(bk_sb[:], bk[:])
            nc.gpsimd.dma_start(bvT_sb[:], bvT[:])
            nc.vector.memset(ones_row[:], 1.0)

            # warm the ACT exp table during the initial DMA window
            nc.vector.memset(warm[:], 1.0)
            nc.scalar.activation(warm[:], warm[:], AF.Exp)

            # vp tiles: [h][tb2] -> [128 keys, 2 key-blocks, 64 d + 1 ones]
            vp_tiles = [[None] * NTB2 for _ in range(2)]
            for tb2 in range(NTB2):
                for h in range(2):
                    vp = vpp.tile([128, 2, 80], fp8, tag="vp",
                                  name=f"vp{h}_{tb2}")
                    nc.vector.memset(vp[:, :, HD:HD + 1], 1.0)
                    vp_tiles[h][tb2] = vp

            with (
                tc.tile_pool(name="w_pool", bufs=1) as wp,
                tc.tile_pool(name="psB", bufs=1, space="PSUM") as psB,
                tc.tile_pool(name="pt_pool", bufs=48) as ptp,
                tc.tile_pool(name="yi_pool", bufs=8) as yip,
                tc.tile_pool(name="small", bufs=3) as sp,
                tc.tile_pool(name="out_pool", bufs=6) as obp,
            ):
                # weights first, one per ring, so no x chunk queues
                # behind more than one weight DMA
                wk_sb = wp.tile([128, 4, FSL], bf16, tag="wk_sb")
                nc.sync.dma_start(wk_sb[:], wkT[:])
                wq_sb = wp.tile([128, 4, FSL], bf16, tag="wq_sb")
                nc.scalar.dma_start(wq_sb[:], wqT[:])
                wv_sb = wp.tile([128, 4, FSL], bf16, tag="wv_sb")
                nc.gpsimd.dma_start(wv_sb[:], wvT[:])

                def load_x(t8):
                    tiles = []
                    cs = slice(t8 * QC, (t8 + 1) * QC)
                    rings = (nc.sync, nc.scalar, nc.gpsimd, nc.sync)
                    for i in range(4):
                        xt_sb = wp.tile([128, QC], bf16, tag="xt", bufs=10,
                                        name="xt_sb")
                        rings[i].dma_start(xt_sb[:],
                                           xT[i * 128:(i + 1) * 128, cs])
                        tiles.append(xt_sb)
                    return tiles

                exp_counter = [0]
                s_counter = [0]

                def emit_attn_pair(qc, h, tb2, ctx_pss):
                    unit = UNIT_PATTERN[exp_counter[0] % 32]
                    exp_counter[0] += 1
                    hh = h * HD
                    pt = ptp.tile([128, 2, QC], fp8, tag="pt", name="pt")
                    if unit == "F":
                        st = psB.tile([128, 2, QC], f32, tag="st", bufs=2,
                                      name="st")
                        for j in range(2):
                            tb = tb2 * 2 + j
                            nc.tensor.matmul(
                                st[:, j, :],
                                kt[hh:hh + HD, tb * 128:(tb + 1) * 128],
                                qt[hh:hh + HD, qc * QC:(qc + 1) * QC],
                                start=True, stop=True)
                        nc.scalar.activation(pt[:], st[:], AF.Exp, scale=SCALE)
                    else:
                        for j in range(2):
                            tb = tb2 * 2 + j
                            sth = psB.tile([128, QC], f32, tag="sth", bufs=2,
                                           name="sth")
                            nc.tensor.matmul(
                                sth[:],
                                kt[hh:hh + HD, tb * 128:(tb + 1) * 128],
                                qt[hh:hh + HD, qc * QC:(qc + 1) * QC],
                                start=True, stop=True)
                            yi = yip.tile([128, QC], i32, tag="yi", name="yi")
                            nc.vector.tensor_scalar(yi[:], sth[:], A_CONST,
                                                    B_CONST, MULT, ADD)
                            s_counter[0] += 1
                            ce = (nc.vector if s_counter[0] % 6 == 0
                                  else nc.gpsimd)
                            ce.tensor_copy(pt[:, j, :], yi[:].bitcast(f32))
                    nc.tensor.matmul(
                        ctx_pss[h][:], vp_tiles[h][tb2][:, :, 0:HD + 1],
                        pt[:],
                        start=(tb2 == 0), stop=(tb2 == NTB2 - 1),
                        perf_mode=DR)

                def emit_attn_tail(qc, ctx_pss):
                    qs = slice(qc * QC, (qc + 1) * QC)
                    for h in range(2):
                        ctx_ps = ctx_pss[h]
                        r_t = sp.tile([1, QC], f32, tag="r", name="r_t")
                        nc.vector.reciprocal_approx_fast(r_t[:],
                                                         ctx_ps[HD:HD + 1, :])
                        rb_sb = sp.tile([HD, QC], f32, tag="rbs", name="rb_sb")
                        nc.gpsimd.partition_broadcast(rb_sb[:], r_t[:])
                        dst = ctx01[0:HD, qs] if h == 0 else ctx1b[:, qs]
                        nc.vector.tensor_mul(dst, ctx_ps[0:HD, :], rb_sb[:])
                    for sc in range(QC // 128):
                        col = qc * QC + sc * 128
                        po = psB.tile([128, HID], f32, tag="ctx", bufs=2,
                                      name="po")
                        nc.tensor.matmul(po[:], ctx01[0:HD, col:col + 128],
                                         wo_sb[0:HD, :],
                                         start=True, stop=False)
                        nc.tensor.matmul(po[:], ctx1b[:, col:col + 128],
                                         wo2_sb[:], start=False, stop=True)
                        ob = obp.tile([128, HID], f32, tag="ob", name="ob")
                        if sc == 0:
                            nc.vector.tensor_copy(ob[:], po[:])
                        else:
                            nc.scalar.copy(ob[:], po[:])
                        nc.sync.dma_start(out[col:col + 128, :], ob[:])

                ctx_q0 = [
                    psB.tile([HD + 1, QC], f32, tag="ctx", bufs=2,
                             name=f"ctxq0{h}")
                    for h in range(2)
                ]
                ctx_q1 = [
                    psB.tile([HD + 1, QC], f32, tag="ctx", bufs=2,
                             name=f"ctxq1{h}")
                    for h in range(2)
                ]
                ctx_q2 = [
                    psB.tile([HD + 1, QC], f32, tag="ctx", bufs=2,
                             name=f"ctxq2{h}")
                    for h in range(2)
                ]
                xcs_next = load_x(0)
                for t8 in range(NQC):
                    cs = slice(t8 * QC, (t8 + 1) * QC)
                    xcs = xcs_next
                    if t8 + 1 < NQC:
                        xcs_next = load_x(t8 + 1)
                    for w_sb, bias_sb, dst in ((wk_sb, bk_sb, kt),
                                               (wq_sb, bq_sb, qt)):
                        ps = psB.tile([FSL, QC], f32, tag="sth", bufs=2,
                                      name="kqps")
                        for i in range(4):
                            nc.tensor.matmul(
                                ps[:], w_sb[:, i, :], xcs[i][:],
                                start=(i == 0), stop=(i == 3))
                        nc.vector.tensor_scalar_add(dst[:, cs], ps[:],
                                                    bias_sb[:])
                    # V in natural [t, d] layout: x^T chunks are stationary,
                    # bias comes from a K=1 ones-row matmul
                    for j in range(4):
                        tb = 4 * t8 + j
                        vps = psB.tile([128, FSL], f32, tag="sth", bufs=2,
                                       name="vps")
                        nc.tensor.matmul(vps[:], ones_row[:], bvT_sb[:],
                                         start=True, stop=False)
                        for i in range(4):
                            nc.tensor.matmul(
                                vps[:], xcs[i][:, j * 128:(j + 1) * 128],
                                wv_sb[:, i, :],
                                start=False, stop=(i == 3))
                        tb2, jj = divmod(tb, 2)
                        for h in range(2):
                            nc.vector.tensor_copy(
                                vp_tiles[h][tb2][:, jj, 0:HD],
                                vps[:, h * HD:(h + 1) * HD])
                    if t8 == 0:
                        # Wo load deferred off the scalar ring's head;
                        # rows 64-127 again at base partition 0 for the
                        # split out-projection
                        nc.scalar.dma_start(wo_sb[:], woT[:])
                        nc.scalar.dma_start(wo2_sb[:], woT[HD:FSL, :])
                    # attention for qc=0 rides along as soon as this t8's
                    # K/Q/V chunks exist
                    for tb2 in (2 * t8, 2 * t8 + 1):
                        for h in range(2):
                            emit_attn_pair(0, h, tb2, ctx_q0)
                    # qc=1/qc=2 ride one/two t8 behind qc=0: scores/exps
                    # run now, their PVs queue until ctx slots free up
                    if t8 >= 1:
                        for tb2e in (2 * (t8 - 1), 2 * (t8 - 1) + 1):
                            for h in range(2):
                                emit_attn_pair(1, h, tb2e, ctx_q1)
                    if t8 >= 2:
                        for tb2e in (2 * (t8 - 2), 2 * (t8 - 2) + 1):
                            for h in range(2):
                                emit_attn_pair(2, h, tb2e, ctx_q2)
                # ---------------- attention + output projection ----------
                # qc=1/2 residuals first, then a software-pipelined loop:
                # the first LEAD units of qc are emitted before the tail of
                # qc-1 so the PE/exp pipeline flows through the serial tail
                LEAD = 10
                for tb2 in range(NTB2 - 2, NTB2):
                    for h in range(2):
                        emit_attn_pair(1, h, tb2, ctx_q1)
                emit_attn_tail(0, ctx_q0)
                for tb2 in range(NTB2 - 4, NTB2):
                    for h in range(2):
                        emit_attn_pair(2, h, tb2, ctx_q2)
                emit_attn_tail(1, ctx_q1)
                prev = (2, ctx_q2)
                for qc in range(3, NQC):
                    ctx_pss = [
                        psB.tile([HD + 1, QC], f32, tag="ctx", bufs=2,
                                 name=f"ctxps{h}")
                        for h in range(2)
                    ]
                    units = [(tb2, h) for tb2 in range(NTB2)
                             for h in range(2)]
                    for tb2, h in units[:LEAD]:
                        emit_attn_pair(qc, h, tb2, ctx_pss)
                    emit_attn_tail(*prev)
                    for tb2, h in units[LEAD:]:
                        emit_attn_pair(qc, h, tb2, ctx_pss)
                    prev = (qc, ctx_pss)
                emit_attn_tail(*prev)

    nc.compile()
    return nc


def _get_program():
    global _PROGRAM
    if _PROGRAM is None:
        _PROGRAM = _build_program()
    return _PROGRAM


def kernel(**inputs):
    _ensure_imports()
    import ml_dtypes
    from concourse import bass_utils

    bf16 = ml_dtypes.bfloat16

    x = np.ascontiguousarray(np.asarray(inputs["x"], dtype=np.float32))
    Wq = np.asarray(inputs["Wq"], dtype=np.float32)
    Wk = np.asarray(inputs["Wk"], dtype=np.float32)
    Wv = np.asarray(inputs["Wv"], dtype=np.float32)
    Wo = np.asarray(inputs["Wo"], dtype=np.float32)
    bq = np.asarray(inputs["bq"], dtype=np.float32)
    bk = np.asarray(inputs["bk"], dtype=np.float32)
    bv = np.asarray(inputs["bv"], dtype=np.float32)
    bo = np.asarray(inputs["bo"], dtype=np.float32)

    nc = _get_program()

    wqT_full = np.ascontiguousarray(Wq.T)
    wkT_full = np.ascontiguousarray(Wk.T)
    wvT_full = np.ascontiguousarray(Wv.T)
    woT_full = np.ascontiguousarray(Wo.T)

    def fold_w(wT_slice):
        # [512, 128] -> [128 partitions, 4 chunks, 128 features] bf16
        return np.ascontiguousarray(
            wT_slice.reshape(4, 128, FSL).transpose(1, 0, 2).astype(bf16))

    in_maps = []
    for c in range(NCORES):
        b, g = divmod(c, 4)
        fs = slice(g * FSL, (g + 1) * FSL)
        in_maps.append({
            "xT": np.ascontiguousarray(x[b].T.astype(bf16)),
            "wqT": fold_w(wqT_full[:, fs]),
            "wkT": fold_w(wkT_full[:, fs]),
            "wvT": fold_w(wvT_full[:, fs]),
            "woT": np.ascontiguousarray(woT_full[fs, :].astype(bf16)),
            "bq": np.ascontiguousarray(bq[fs].reshape(FSL, 1)),
            "bk": np.ascontiguousarray(bk[fs].reshape(FSL, 1)),
            "bvT": np.ascontiguousarray(bv[fs].reshape(1, FSL).astype(bf16)),
        })

    res = bass_utils.run_bass_kernel_spmd(nc, in_maps,
                                          core_ids=list(range(NCORES)))
    outs = [r["out"] for r in res.results]

    full = np.empty((B, S, HID), dtype=np.float32)
    for b in range(B):
        full[b] = outs[4 * b] + outs[4 * b + 1] + outs[4 * b + 2] + outs[4 * b + 3]
        full[b] += bo
    return full
